# revision 1
# baseline (speedup 1.0000x reference)
"""Causal self-attention (B=4, T=2048, C=1024, NH=16) on 8 TRN2 NeuronCores.

Sharding: core = 2*b + g  (b in 0..3 batches, g in 0..1 head-groups of 8 heads).
Each core computes qkv projection for its 8 heads, causal flash attention,
and a partial output projection (rows g*512:(g+1)*512 of w_proj).  Host sums
the two partials per batch and adds b_proj.

Layouts on device (per core):
  qT, kT : [head-dims on partitions, T on free]  (from  W.T @ x.T  matmuls)
  v      : natural [T on partitions, head-dims on free], with a ones-column
           appended per head so the PV matmul also produces the softmax
           denominator (lhsT = [v_h | 1] -> out rows 0..63 = y^T, row 64 = sum)
  S^T    : [keys on partitions, queries on free]; exp on ScalarE (no max
           subtraction needed: |S/8| <~ 6 for N(0,1) logits), causal mask via
           gpsimd affine_select on the 4 diagonal tiles per query block.
"""

import numpy as np

import concourse.bass as bass
import concourse.mybir as mybir
import concourse.tile as tile
from concourse import bacc
from concourse.bass_utils import run_bass_kernel_spmd
from concourse.masks import make_identity

B, T, C = 4, 2048, 1024
NH, HD = 16, 64
G = 2              # head groups (cores per batch)
HPG = NH // G      # heads per group = 8
GD = HPG * HD      # dims per group = 512
N_CORES = B * G

FP32 = mybir.dt.float32

# matmul dtype mode: "f32" (exact, 4 cyc/row), "f32r" (1 cyc/row at N>=256),
# "bf16" (1 cyc/row, operands stored bf16)
MM_MODE = "bf16"


def _st_dt():
    """dtype of SBUF tiles that feed TensorE matmuls (walrus requires
    float32r-producing instructions for fp32r matmul operands)."""
    if MM_MODE == "bf16":
        return mybir.dt.bfloat16
    if MM_MODE == "f32r":
        return mybir.dt.float32r
    return FP32


def _xn_dt():
    """dtype of the x-natural tiles / PE-transpose path (plain f32 there)."""
    return mybir.dt.bfloat16 if MM_MODE == "bf16" else FP32


def _mm(ap):
    return ap


def build_nc():
    ST = _st_dt()
    XN = _xn_dt()
    nc = bacc.Bacc()

    x = nc.declare_dram_parameter("x", [T, C], XN, isOutput=False)
    wq = nc.declare_dram_parameter("wq", [C, GD], ST, isOutput=False)
    wk = nc.declare_dram_parameter("wk", [C, GD], ST, isOutput=False)
    wv = nc.declare_dram_parameter("wv", [C, GD], ST, isOutput=False)
    bq = nc.declare_dram_parameter("bq", [GD], FP32, isOutput=False)
    bk = nc.declare_dram_parameter("bk", [GD], FP32, isOutput=False)
    bv = nc.declare_dram_parameter("bv", [GD], FP32, isOutput=False)
    wp = nc.declare_dram_parameter("wp", [GD, C], ST, isOutput=False)
    ones = nc.declare_dram_parameter("ones", [128, HPG], ST, isOutput=False)
    out = nc.declare_dram_parameter("out", [T, C], FP32, isOutput=True)

    NCC = C // 128      # 8 contraction chunks for the qkv projection
    NMB = GD // 128     # 4 blocks of 128 qkv-dims per section
    NTB = T // 512      # 4 T-blocks of 512
    NKC = T // 128      # 16 key chunks of 128

    from contextlib import ExitStack

    with tile.TileContext(nc) as tc, ExitStack() as stack:
        consts = stack.enter_context(tc.tile_pool(name="consts", bufs=1))
        persist = stack.enter_context(tc.tile_pool(name="persist", bufs=1))

        if MM_MODE != "bf16":
            ident = consts.tile([128, 128], XN, tag="ident")
            make_identity(nc, ident)
        bq_col = consts.tile([128, NMB], FP32, tag="bq_col")
        bk_col = consts.tile([128, NMB], FP32, tag="bk_col")
        for m in range(NMB):
            nc.sync.dma_start(out=bq_col[:, m : m + 1], in_=bq[bass.ts(m, 128)])
            nc.sync.dma_start(out=bk_col[:, m : m + 1], in_=bk[bass.ts(m, 128)])
        # bv broadcast to all 128 partitions (DMA supports partition step 0)
        bv_bc = consts.tile([128, GD], FP32, tag="bv_bc")
        nc.sync.dma_start(out=bv_bc, in_=bv[None, :].partition_broadcast(128))

        # persistent activations
        qT_t = [persist.tile([128, T], ST, tag=f"qT{m}", name=f"qT{m}") for m in range(NMB)]
        kT_t = [persist.tile([128, T], ST, tag=f"kT{m}", name=f"kT{m}") for m in range(NMB)]
        v_all = persist.tile([128, NKC, HPG, HD + 1], ST, tag="v_all", name="v_all")
        # y^T reuses the qT tiles: the query columns of head-pair m, block qb
        # are dead once that block's PV matmuls have consumed them.
        yT_t = qT_t

        # ---------------- Stage A: qkv projection ----------------
        with (
            tc.tile_pool(name="wA", bufs=1) as wA_pool,
            tc.tile_pool(name="xA", bufs=3) as xA_pool,
            tc.tile_pool(name="xT", bufs=8) as xT_pool,
            tc.tile_pool(name="trps", bufs=2, space="PSUM") as trps_pool,
            tc.tile_pool(name="qkvps", bufs=4, space="PSUM") as qkvps_pool,
        ):
            wq_t = wA_pool.tile([128, NCC, GD], ST, tag="wq")
            wk_t = wA_pool.tile([128, NCC, GD], ST, tag="wk")
            wv_t = wA_pool.tile([128, NCC, GD], ST, tag="wv")
            for c in range(NCC):
                nc.sync.dma_start(out=wq_t[:, c, :], in_=wq[bass.ts(c, 128), :])
                nc.sync.dma_start(out=wk_t[:, c, :], in_=wk[bass.ts(c, 128), :])
                nc.sync.dma_start(out=wv_t[:, c, :], in_=wv[bass.ts(c, 128), :])

            for tb in range(NTB):
                # x^T chunks for this block of 512 timesteps
                xtc = [xT_pool.tile([128, 512], ST, tag="xtc", name="xtc") for _ in range(NCC)]
                if MM_MODE == "bf16":
                    # 2-byte dtype: hardware xbar DMA transpose straight from DRAM
                    for c in range(NCC):
                        nc.sync.dma_start(
                            out=xtc[c],
                            in_=x[bass.ts(tb, 512), bass.ts(c, 128)],
                            transpose=True,
                        )
                else:
                    for tsub in range(4):
                        xn = xA_pool.tile([128, C], XN, tag="xn")
                        t0 = tb * 512 + tsub * 128
                        nc.sync.dma_start(out=xn, in_=x[t0 : t0 + 128, :])
                        for c in range(NCC):
                            trp = trps_pool.tile([128, 128], XN, tag="trp")
                            nc.tensor.transpose(trp, xn[:, bass.ts(c, 128)], ident)
                            nc.vector.tensor_copy(
                                xtc[c][:, bass.ts(tsub, 128)], trp
                            )

                # q^T and k^T for this T-block
                for w_t, b_col, dst in ((wq_t, bq_col, qT_t), (wk_t, bk_col, kT_t)):
                    for m in range(NMB):
                        ps = qkvps_pool.tile([128, 512], FP32, tag="qkvps")
                        for c in range(NCC):
                            nc.tensor.matmul(
                                ps,
                                _mm(w_t[:, c, bass.ts(m, 128)]),
                                _mm(xtc[c]),
                                start=(c == 0),
                                stop=(c == NCC - 1),
                            )
                        nc.scalar.activation(
                            out=dst[m][:, bass.ts(tb, 512)],
                            in_=ps,
                            func=mybir.ActivationFunctionType.Identity,
                            bias=b_col[:, m : m + 1],
                        )

                # v natural for this T-block (4 key chunks of 128)
                for tsub in range(4):
                    kc = tb * 4 + tsub
                    ps = qkvps_pool.tile([128, GD], FP32, tag="qkvps")
                    for c in range(NCC):
                        nc.tensor.matmul(
                            ps,
                            _mm(xtc[c][:, bass.ts(tsub, 128)]),
                            _mm(wv_t[:, c, :]),
                            start=(c == 0),
                            stop=(c == NCC - 1),
                        )
                    vt = v_all[:, kc, :, :]
                    nc.vector.tensor_add(
                        vt[:, :, 0:HD],
                        ps.rearrange("p (h d) -> p h d", h=HPG),
                        bv_bc.rearrange("p (h d) -> p h d", h=HPG),
                    )
                    nc.sync.dma_start(
                        out=vt[:, :, HD : HD + 1], in_=ones[:, :, None]
                    )

        # ---------------- Stage B: causal attention + interleaved proj ----------------
        PDT = _st_dt()
        with (
            tc.tile_pool(name="pT", bufs=10) as pT_pool,
            tc.tile_pool(name="rec", bufs=4) as rec_pool,
            tc.tile_pool(name="wp", bufs=1) as wp_pool,
            tc.tile_pool(name="osb", bufs=4) as osb_pool,
            tc.tile_pool(name="sps", bufs=3, space="PSUM") as sps_pool,
            tc.tile_pool(name="pvps", bufs=3, space="PSUM") as pvps_pool,
            tc.tile_pool(name="ops", bufs=2, space="PSUM") as ops_pool,
        ):
            wp_t = wp_pool.tile([128, NMB, C], ST, tag="wp")
            for c in range(NMB):
                nc.sync.dma_start(out=wp_t[:, c, :], in_=wp[bass.ts(c, 128), :])

            dens = {}
            for qb in range(NTB):
                kcmax = (qb + 1) * 4
                for m in range(NMB):
                    if qb == 0:
                        dens[m] = rec_pool.tile(
                            [64, 512], FP32, tag=f"den{m}", name=f"den{m}", bufs=1
                        )
                        nc.vector.memset(dens[m], 1.0)
                    den = dens[m]
                    pvs = [
                        pvps_pool.tile([HD + 1, 512], FP32, tag="pvps", name="pvps")
                        for _ in range(2)
                    ]
                    for kc in range(kcmax):
                        pTs = []
                        for hp in range(2):
                            base = hp * 64
                            sp = sps_pool.tile([128, 512], FP32, tag="sps")
                            nc.tensor.matmul(
                                sp,
                                _mm(kT_t[m][base : base + 64, bass.ts(kc, 128)]),
                                _mm(qT_t[m][base : base + 64, bass.ts(qb, 512)]),
                                start=True,
                                stop=True,
                            )
                            pT = pT_pool.tile([128, 512], PDT, tag="pT")
                            nc.scalar.activation(
                                out=pT,
                                in_=sp,
                                func=mybir.ActivationFunctionType.Exp,
                                scale=1.0 / float(np.sqrt(HD)),
                            )
                            r = kc - qb * 4
                            if r >= 0:
                                # keep key j <= query i:  (il - jl - 128 r) >= 0
                                nc.gpsimd.affine_select(
                                    out=pT,
                                    in_=pT,
                                    compare_op=mybir.AluOpType.is_ge,
                                    fill=0.0,
                                    base=-128 * r,
                                    channel_multiplier=-1,
                                    pattern=[[1, 512]],
                                )
                            pTs.append(pT)
                        for hp in range(2):
                            h = 2 * m + hp
                            nc.tensor.matmul(
                                pvs[hp],
                                _mm(v_all[:, kc, h, :]),
                                _mm(pTs[hp]),
                                start=(kc == 0),
                                stop=(kc == kcmax - 1),
                            )
                    for hp in range(2):
                        base = hp * 64
                        # unnormalized y^T and denominator row; normalize below
                        nc.vector.tensor_copy(
                            yT_t[m][base : base + 64, bass.ts(qb, 512)],
                            pvs[hp][0:HD, :],
                        )
                        nc.vector.tensor_copy(
                            den[32 * hp : 32 * hp + 1, :],
                            pvs[hp][HD : HD + 1, :],
                        )
                # normalize all head-pairs for this qb (batched reciprocal per m
                # amortizes the DVE per-free-element reciprocal cost)
                for m in range(NMB):
                    den = dens[m]
                    denr = rec_pool.tile([64, 512], FP32, tag=f"denr{m}", name=f"denr{m}", bufs=2)
                    nc.vector.reciprocal(denr, den)
                    for hp in range(2):
                        base = hp * 64
                        if hp == 0:
                            src_row = denr[0:1, :]
                        else:
                            dtmp = rec_pool.tile([1, 512], FP32, tag="dtmp", name="dtmp")
                            nc.vector.tensor_copy(dtmp, denr[32:33, :])
                            src_row = dtmp
                        rbc = rec_pool.tile([128, 512], FP32, tag="rbc", name="rbc")
                        nc.gpsimd.partition_broadcast(rbc, src_row)
                        nc.vector.tensor_mul(
                            yT_t[m][base : base + 64, bass.ts(qb, 512)],
                            yT_t[m][base : base + 64, bass.ts(qb, 512)],
                            rbc[base : base + 64, :],
                        )
                # output projection for the 4 T-subblocks of this query block
                for tsub in range(4):
                    tb16 = qb * 4 + tsub
                    for nb in range(C // 512):
                        ps = ops_pool.tile([128, 512], FP32, tag="ops")
                        for c in range(NMB):
                            nc.tensor.matmul(
                                ps,
                                _mm(yT_t[c][:, bass.ts(tb16, 128)]),
                                _mm(wp_t[:, c, bass.ts(nb, 512)]),
                                start=(c == 0),
                                stop=(c == NMB - 1),
                            )
                        osb = osb_pool.tile([128, 512], FP32, tag="osb")
                        nc.scalar.copy(osb, ps)
                        nc.sync.dma_start(
                            out=out[bass.ts(tb16, 128), bass.ts(nb, 512)], in_=osb
                        )

    nc.compile()
    return nc


_CACHE = {}


def _get_nc():
    if "nc" not in _CACHE:
        _CACHE["nc"] = build_nc()
    return _CACHE["nc"]


def _to_st(a):
    a = np.asarray(a, dtype=np.float32)
    if MM_MODE == "bf16":
        import ml_dtypes

        return np.ascontiguousarray(a.astype(ml_dtypes.bfloat16))
    return np.ascontiguousarray(a)


def make_in_maps(x, w_qkv, b_qkv, w_proj):
    x = np.asarray(x, dtype=np.float32)
    w_qkv = np.asarray(w_qkv, dtype=np.float32)
    b_qkv = np.asarray(b_qkv, dtype=np.float32)
    in_maps = []
    for core in range(N_CORES):
        b, g = divmod(core, G)
        in_maps.append(
            {
                "x": _to_st(x[b]),
                "wq": _to_st(w_qkv[:, GD * g : GD * g + GD]),
                "wk": _to_st(w_qkv[:, C + GD * g : C + GD * g + GD]),
                "wv": _to_st(w_qkv[:, 2 * C + GD * g : 2 * C + GD * g + GD]),
                "bq": np.ascontiguousarray(b_qkv[GD * g : GD * g + GD]),
                "bk": np.ascontiguousarray(b_qkv[C + GD * g : C + GD * g + GD]),
                "bv": np.ascontiguousarray(b_qkv[2 * C + GD * g : 2 * C + GD * g + GD]),
                "wp": _to_st(np.asarray(w_proj, dtype=np.float32)[GD * g : GD * g + GD, :]),
                "ones": _to_st(np.ones((128, HPG), dtype=np.float32)),
            }
        )
    return in_maps


def _assemble(results, b_proj):
    y = np.empty((B, T, C), dtype=np.float32)
    for b in range(B):
        y[b] = results[G * b]["out"] + results[G * b + 1]["out"]
    y += np.asarray(b_proj, dtype=np.float32)[None, None, :]
    return y


def kernel(x, w_qkv, b_qkv, w_proj, b_proj):
    nc = _get_nc()
    in_maps = make_in_maps(x, w_qkv, b_qkv, w_proj)
    res = run_bass_kernel_spmd(nc, in_maps, list(range(N_CORES)))
    return _assemble(res.results, b_proj)



# revision 7
# speedup vs baseline: 1.2827x; 1.2827x over previous
"""Causal self-attention (B=4, T=2048, C=1024, NH=16) on 8 TRN2 NeuronCores.

Sharding: core = 2*b + g  (b in 0..3 batches, g in 0..1 head-groups of 8 heads).
Each core computes the qkv projection for its 8 heads, causal flash attention,
and a partial output projection (rows g*512:(g+1)*512 of w_proj).  Host sums
the two partials per batch and adds b_proj.

v2 layout notes (all matmuls full-array K=128/M=128 to keep the PE HAM
activity monitor un-throttled at 2.4 GHz):
  qTp[hp][m] : [128, T] bf16, rows hp*64..hp*64+64 hold head 2m+hp's q^T;
               the other 64 rows are ZERO.  QK matmul uses the full shared
               kT[m] [128, T] as stationary (both heads' dims) and qTp[hp]
               as moving: the zero rows null the other head's contribution,
               so out = S^T for head 2m+hp with K=128 rows streaming.
  v_pad      : [128, kc, h, 128] bf16 = [64 v-dims | ones | 63 zeros] so the
               PV matmul is K=128/M=128; out row 64 = softmax denominator.
  exp        : one ACT call per TWO key chunks ([128,1024] across 2 PSUM
               banks) to amortize the 352-cycle ACT startup.
  softmax div: denominators gathered into den8 [8,512], one
               reciprocal_approx_fast per query block, gpsimd broadcast,
               in-place bf16 multiply on the unnormalized y^T.
  proj       : lags one query block behind attention so the PE never waits
               on the normalization chain; outputs staged via DVE copy.
"""

import numpy as np

import concourse.bass as bass
import concourse.mybir as mybir
import concourse.tile as tile
from concourse import bacc
from concourse.bass_utils import run_bass_kernel_spmd

B, T, C = 4, 2048, 1024
NH, HD = 16, 64
G = 2              # head groups (cores per batch)
HPG = NH // G      # heads per group = 8
GD = HPG * HD      # dims per group = 512
N_CORES = B * G

FP32 = mybir.dt.float32
BF16 = mybir.dt.bfloat16

NCC = C // 128      # 8 contraction chunks for the qkv projection
NMB = GD // 128     # 4 blocks of 128 qkv-dims per section (head pairs)
NTB = T // 512      # 4 T-blocks of 512
NKC = T // 128      # 16 key chunks of 128


def build_nc():
    nc = bacc.Bacc()

    x = nc.declare_dram_parameter("x", [T, C], BF16, isOutput=False)
    wq = nc.declare_dram_parameter("wq", [C, GD], BF16, isOutput=False)
    wk = nc.declare_dram_parameter("wk", [C, GD], BF16, isOutput=False)
    wv = nc.declare_dram_parameter("wv", [C, GD], BF16, isOutput=False)
    bq = nc.declare_dram_parameter("bq", [GD], FP32, isOutput=False)
    bk = nc.declare_dram_parameter("bk", [GD], FP32, isOutput=False)
    bv = nc.declare_dram_parameter("bv", [GD], FP32, isOutput=False)
    wp = nc.declare_dram_parameter("wp", [GD, C], BF16, isOutput=False)
    ones = nc.declare_dram_parameter("ones", [128, HPG], BF16, isOutput=False)
    out = nc.declare_dram_parameter("out", [T, C], FP32, isOutput=True)

    from contextlib import ExitStack

    with tile.TileContext(nc) as tc, ExitStack() as stack:
        consts = stack.enter_context(tc.tile_pool(name="consts", bufs=1))
        persist = stack.enter_context(tc.tile_pool(name="persist", bufs=1))

        # ---- persistent activations ----
        qTp = [
            [persist.tile([128, T], BF16, tag=f"qTp{hp}{m}", name=f"qTp{hp}{m}")
             for m in range(NMB)]
            for hp in range(2)
        ]
        kT_t = [persist.tile([128, T], BF16, tag=f"kT{m}", name=f"kT{m}")
                for m in range(NMB)]
        yT_t = [persist.tile([128, T], BF16, tag=f"yT{m}", name=f"yT{m}")
                for m in range(NMB)]
        v_pad = persist.tile([128, NKC, HPG, 128], BF16, tag="v_pad", name="v_pad")
        xtc = [
            [persist.tile([128, 512], BF16, tag=f"xtc{tb}_{c}", name=f"xtc{tb}_{c}")
             for c in range(NCC)]
            for tb in range(NTB)
        ]
        wq_c = [persist.tile([128, GD], BF16, tag=f"wq{c}", name=f"wq{c}")
                for c in range(NCC)]
        wk_c = [persist.tile([128, GD], BF16, tag=f"wk{c}", name=f"wk{c}")
                for c in range(NCC)]
        wv_c = [persist.tile([128, GD], BF16, tag=f"wv{c}", name=f"wv{c}")
                for c in range(NCC)]
        wp_m = [persist.tile([128, C], BF16, tag=f"wp{m}", name=f"wp{m}")
                for m in range(NMB)]

        # ---- zero-fill the pad regions (overlaps the initial DMAs) ----
        for hp in range(2):
            zbase = (1 - hp) * 64
            for m in range(NMB):
                nc.vector.memset(qTp[hp][m][zbase : zbase + 64, :], 0.0)
        nc.vector.memset(v_pad[:, :, :, HD + 1 :], 0.0)

        # ---- input DMAs (x^T chunks first so stage A can start early) ----
        for c in range(NCC):
            nc.sync.dma_start(
                out=xtc[0][c], in_=x[0:512, bass.ts(c, 128)], transpose=True
            )
            nc.sync.dma_start(out=wq_c[c], in_=wq[bass.ts(c, 128), :])
            nc.sync.dma_start(out=wk_c[c], in_=wk[bass.ts(c, 128), :])
            nc.sync.dma_start(out=wv_c[c], in_=wv[bass.ts(c, 128), :])
        for tb in range(1, NTB):
            for c in range(NCC):
                nc.sync.dma_start(
                    out=xtc[tb][c],
                    in_=x[bass.ts(tb, 512), bass.ts(c, 128)],
                    transpose=True,
                )
        for m in range(NMB):
            nc.sync.dma_start(out=wp_m[m], in_=wp[bass.ts(m, 128), :])

        bq_col = consts.tile([128, NMB], FP32, tag="bq_col")
        bk_col = consts.tile([128, NMB], FP32, tag="bk_col")
        for m in range(NMB):
            nc.sync.dma_start(out=bq_col[:, m : m + 1], in_=bq[bass.ts(m, 128)])
            nc.sync.dma_start(out=bk_col[:, m : m + 1], in_=bk[bass.ts(m, 128)])
        bv_bc = consts.tile([128, GD], FP32, tag="bv_bc")
        nc.sync.dma_start(out=bv_bc, in_=bv[None, :].partition_broadcast(128))
        # ones column of v_pad (after the memset in program order)
        for kc in range(NKC):
            nc.sync.dma_start(out=v_pad[:, kc, :, HD : HD + 1], in_=ones[:, :, None])

        # ---------------- Stage A: qkv projection ----------------
        with tc.tile_pool(name="qkvps", bufs=4, space="PSUM") as qkvps_pool:
            for tb in range(NTB):
                tcols = bass.ts(tb, 512)
                for m in range(NMB):
                    ps = qkvps_pool.tile([128, 512], FP32, tag="qkvps")
                    for c in range(NCC):
                        nc.tensor.matmul(
                            ps,
                            wq_c[c][:, bass.ts(m, 128)],
                            xtc[tb][c],
                            start=(c == 0),
                            stop=(c == NCC - 1),
                        )
                    nc.vector.tensor_scalar_add(
                        qTp[0][m][0:64, tcols], ps[0:64, :], bq_col[0:64, m : m + 1]
                    )
                    nc.vector.tensor_scalar_add(
                        qTp[1][m][64:128, tcols], ps[64:128, :], bq_col[64:128, m : m + 1]
                    )
                for m in range(NMB):
                    ps = qkvps_pool.tile([128, 512], FP32, tag="qkvps")
                    for c in range(NCC):
                        nc.tensor.matmul(
                            ps,
                            wk_c[c][:, bass.ts(m, 128)],
                            xtc[tb][c],
                            start=(c == 0),
                            stop=(c == NCC - 1),
                        )
                    nc.vector.tensor_scalar_add(
                        kT_t[m][:, tcols], ps, bk_col[:, m : m + 1]
                    )
                for tsub in range(4):
                    kc = tb * 4 + tsub
                    ps = qkvps_pool.tile([128, GD], FP32, tag="qkvps")
                    for c in range(NCC):
                        nc.tensor.matmul(
                            ps,
                            xtc[tb][c][:, bass.ts(tsub, 128)],
                            wv_c[c],
                            start=(c == 0),
                            stop=(c == NCC - 1),
                        )
                    vt = v_pad[:, kc, :, :]
                    nc.vector.tensor_add(
                        vt[:, :, 0:HD],
                        ps.rearrange("p (h d) -> p h d", h=HPG),
                        bv_bc.rearrange("p (h d) -> p h d", h=HPG),
                    )

        # ---------------- Stage B: causal attention + lagged proj ----------------
        with (
            tc.tile_pool(name="pT", bufs=6) as pT_pool,
            tc.tile_pool(name="rec", bufs=2) as rec_pool,
            tc.tile_pool(name="rbc", bufs=2) as rbc_pool,
            tc.tile_pool(name="osb", bufs=2) as osb_pool,
            tc.tile_pool(name="sps", bufs=2, space="PSUM") as sps_pool,
            tc.tile_pool(name="pvps", bufs=3, space="PSUM") as pvps_pool,
            tc.tile_pool(name="ops", bufs=1, space="PSUM") as ops_pool,
        ):
            dens = {}

            def emit_attention(qb):
                kcmax = (qb + 1) * 4
                for m in range(NMB):
                    for hp in range(2):
                        pv = pvps_pool.tile([128, 512], FP32, tag="pvps", name="pvps")
                        for p in range(kcmax // 2):
                            sp = sps_pool.tile([128, 2, 512], FP32, tag="sps")
                            for j in range(2):
                                kc = 2 * p + j
                                nc.tensor.matmul(
                                    sp[:, j, :],
                                    kT_t[m][:, bass.ts(kc, 128)],
                                    qTp[hp][m][:, bass.ts(qb, 512)],
                                    start=True,
                                    stop=True,
                                )
                            pT2 = pT_pool.tile([128, 1024], BF16, tag="pT2")
                            nc.scalar.activation(
                                out=pT2,
                                in_=sp.rearrange("p a b -> p (a b)"),
                                func=mybir.ActivationFunctionType.Exp,
                                scale=1.0 / float(np.sqrt(HD)),
                            )
                            for j in range(2):
                                kc = 2 * p + j
                                r = kc - qb * 4
                                if r >= 0:
                                    # keep key j <= query i: (jl - il - 128 r) >= 0
                                    nc.gpsimd.affine_select(
                                        out=pT2[:, bass.ts(j, 512)],
                                        in_=pT2[:, bass.ts(j, 512)],
                                        compare_op=mybir.AluOpType.is_ge,
                                        fill=0.0,
                                        base=-128 * r,
                                        channel_multiplier=-1,
                                        pattern=[[1, 512]],
                                    )
                            for j in range(2):
                                kc = 2 * p + j
                                nc.tensor.matmul(
                                    pv,
                                    v_pad[:, kc, 2 * m + hp, :],
                                    pT2[:, bass.ts(j, 512)],
                                    start=(kc == 0),
                                    stop=(kc == kcmax - 1),
                                )
                        if qb == 0 and hp == 0:
                            dens[m] = rec_pool.tile(
                                [64, 512], FP32, tag=f"den{m}", name=f"den{m}", bufs=1
                            )
                            nc.vector.memset(dens[m], 1.0)
                        # unnormalized y^T (bf16) + denominator row (at 0 / 32)
                        nc.vector.tensor_copy(
                            yT_t[m][bass.ts(hp, 64), bass.ts(qb, 512)], pv[0:HD, :]
                        )
                        nc.vector.tensor_copy(
                            dens[m][32 * hp : 32 * hp + 1, :], pv[HD : HD + 1, :]
                        )
                # normalize: cheap approx reciprocal, broadcast, in-place multiply
                for m in range(NMB):
                    denr = rec_pool.tile([64, 512], FP32, tag="denr", name="denr")
                    nc.vector.reciprocal_approx_fast(denr, dens[m])
                    for hp in range(2):
                        if hp == 0:
                            src_row = denr[0:1, :]
                        else:
                            dtmp = rec_pool.tile([1, 512], FP32, tag="dtmp", name="dtmp")
                            nc.vector.tensor_copy(dtmp, denr[32:33, :])
                            src_row = dtmp
                        rbc = rbc_pool.tile([128, 512], FP32, tag="rbc", name="rbc")
                        nc.gpsimd.partition_broadcast(rbc, src_row)
                        yt = yT_t[m][bass.ts(hp, 64), bass.ts(qb, 512)]
                        nc.vector.tensor_mul(yt, yt, rbc[bass.ts(hp, 64), :])

            def emit_proj(qb):
                for tsub in range(4):
                    tb16 = qb * 4 + tsub
                    for nb in range(C // 512):
                        ps = ops_pool.tile([128, 512], FP32, tag="ops")
                        for c in range(NMB):
                            nc.tensor.matmul(
                                ps,
                                yT_t[c][:, bass.ts(tb16, 128)],
                                wp_m[c][:, bass.ts(nb, 512)],
                                start=(c == 0),
                                stop=(c == NMB - 1),
                            )
                        osb = osb_pool.tile([128, 512], FP32, tag="osb")
                        nc.vector.tensor_copy(osb, ps)
                        nc.sync.dma_start(
                            out=out[bass.ts(tb16, 128), bass.ts(nb, 512)], in_=osb
                        )

            # proj lags one query block so the PE never waits on normalization
            emit_attention(0)
            emit_attention(1)
            emit_proj(0)
            emit_attention(2)
            emit_proj(1)
            emit_attention(3)
            emit_proj(2)
            emit_proj(3)

    nc.compile()
    return nc


_CACHE = {}


def _get_nc():
    if "nc" not in _CACHE:
        _CACHE["nc"] = build_nc()
    return _CACHE["nc"]


def _to_bf16(a):
    import ml_dtypes

    a = np.asarray(a, dtype=np.float32)
    return np.ascontiguousarray(a.astype(ml_dtypes.bfloat16))


def make_in_maps(x, w_qkv, b_qkv, w_proj):
    x = np.asarray(x, dtype=np.float32)
    w_qkv = np.asarray(w_qkv, dtype=np.float32)
    b_qkv = np.asarray(b_qkv, dtype=np.float32)
    in_maps = []
    for core in range(N_CORES):
        b, g = divmod(core, G)
        in_maps.append(
            {
                "x": _to_bf16(x[b]),
                "wq": _to_bf16(w_qkv[:, GD * g : GD * g + GD]),
                "wk": _to_bf16(w_qkv[:, C + GD * g : C + GD * g + GD]),
                "wv": _to_bf16(w_qkv[:, 2 * C + GD * g : 2 * C + GD * g + GD]),
                "bq": np.ascontiguousarray(b_qkv[GD * g : GD * g + GD]),
                "bk": np.ascontiguousarray(b_qkv[C + GD * g : C + GD * g + GD]),
                "bv": np.ascontiguousarray(b_qkv[2 * C + GD * g : 2 * C + GD * g + GD]),
                "wp": _to_bf16(np.asarray(w_proj, dtype=np.float32)[GD * g : GD * g + GD, :]),
                "ones": _to_bf16(np.ones((128, HPG), dtype=np.float32)),
            }
        )
    return in_maps


def _assemble(results, b_proj):
    y = np.empty((B, T, C), dtype=np.float32)
    for b in range(B):
        y[b] = results[G * b]["out"] + results[G * b + 1]["out"]
    y += np.asarray(b_proj, dtype=np.float32)[None, None, :]
    return y


def kernel(x, w_qkv, b_qkv, w_proj, b_proj):
    nc = _get_nc()
    in_maps = make_in_maps(x, w_qkv, b_qkv, w_proj)
    res = run_bass_kernel_spmd(nc, in_maps, list(range(N_CORES)))
    return _assemble(res.results, b_proj)


# revision 10
# speedup vs baseline: 1.2882x; 1.0043x over previous
"""Causal self-attention (B=4, T=2048, C=1024, NH=16) on 8 TRN2 NeuronCores.

Sharding: core = 2*b + g  (b in 0..3 batches, g in 0..1 head-groups of 8 heads).
Each core computes the qkv projection for its 8 heads, causal flash attention,
and a partial output projection (rows g*512:(g+1)*512 of w_proj).  Host sums
the two partials per batch and adds b_proj.

v3 design (every matmul is K=128/M=128/N=512 so the PE HAM activity monitor
keeps the array at the warm 2.4 GHz clock):
  x^T     : x is DMA'd linearly and transposed on the PE in a prologue
            (DMA-transpose of bf16 measured ~40 GB/s — far too slow).
  qTp     : [2][m][128, T] bf16; rows hp*64..+64 hold head 2m+hp's q^T, the
            other 64 rows stay zero.  QK uses the shared kT[m] (both heads'
            dims) as stationary; the zero q rows null the other head.
  v_pad   : [128, kc, h, 128] bf16 = [64 v-dims | ones | 63 zeros]; PV output
            row 64 is the softmax denominator (ones-column trick).
  exp     : one ACT call per TWO key chunks ([128,1024] across 2 PSUM banks)
            to amortize the 352-cycle ACT startup; causal mask applied with
            one fused gpsimd affine_select per diagonal pair.
  softmax : reciprocal_approx_fast on [64,512] den tiles (rows 0/32), gpsimd
            partition broadcast, in-place bf16 multiply on unnormalized y^T.
  schedule: qkv(tb) / attention(qb) / proj(qb-1) interleaved; one shared
            [128,2,512] PSUM pool serves both the qkv pairs and the S pairs
            (4 banks) + 3 PV banks + 1 proj bank = 8.
"""

import numpy as np

import concourse.bass as bass
import concourse.mybir as mybir
import concourse.tile as tile
from concourse import bacc
from concourse.bass_utils import run_bass_kernel_spmd
from concourse.masks import make_identity

B, T, C = 4, 2048, 1024
NH, HD = 16, 64
G = 2              # head groups (cores per batch)
HPG = NH // G      # heads per group = 8
GD = HPG * HD      # dims per group = 512
N_CORES = B * G

FP32 = mybir.dt.float32
BF16 = mybir.dt.bfloat16

NCC = C // 128      # 8 contraction chunks for the qkv projection
NMB = GD // 128     # 4 blocks of 128 qkv-dims per section (head pairs)
NTB = T // 512      # 4 T-blocks of 512
NKC = T // 128      # 16 key chunks of 128


def build_nc():
    nc = bacc.Bacc()

    x = nc.declare_dram_parameter("x", [T, C], BF16, isOutput=False)
    wq = nc.declare_dram_parameter("wq", [C, GD], BF16, isOutput=False)
    wk = nc.declare_dram_parameter("wk", [C, GD], BF16, isOutput=False)
    wv = nc.declare_dram_parameter("wv", [C, GD], BF16, isOutput=False)
    bq = nc.declare_dram_parameter("bq", [GD], FP32, isOutput=False)
    bk = nc.declare_dram_parameter("bk", [GD], FP32, isOutput=False)
    bv = nc.declare_dram_parameter("bv", [GD], FP32, isOutput=False)
    wp = nc.declare_dram_parameter("wp", [GD, C], BF16, isOutput=False)
    ones = nc.declare_dram_parameter("ones", [128, HPG], BF16, isOutput=False)
    out = nc.declare_dram_parameter("out", [T, C], FP32, isOutput=True)

    from contextlib import ExitStack

    with tile.TileContext(nc) as tc, ExitStack() as stack:
        consts = stack.enter_context(tc.tile_pool(name="consts", bufs=1))
        persist = stack.enter_context(tc.tile_pool(name="persist", bufs=1))

        # ---- persistent activations ----
        qTp = [
            [persist.tile([128, T], BF16, tag=f"qTp{hp}{m}", name=f"qTp{hp}{m}")
             for m in range(NMB)]
            for hp in range(2)
        ]
        kT_t = [persist.tile([128, T], BF16, tag=f"kT{m}", name=f"kT{m}")
                for m in range(NMB)]
        yT_t = [persist.tile([128, T], BF16, tag=f"yT{m}", name=f"yT{m}")
                for m in range(NMB)]
        v_pad = persist.tile([128, NKC, HPG, 128], BF16, tag="v_pad", name="v_pad")
        xtc = [
            [persist.tile([128, 512], BF16, tag=f"xtc{tb}_{c}", name=f"xtc{tb}_{c}")
             for c in range(NCC)]
            for tb in range(NTB)
        ]
        wq_c = [persist.tile([128, GD], BF16, tag=f"wq{c}", name=f"wq{c}")
                for c in range(NCC)]
        wk_c = [persist.tile([128, GD], BF16, tag=f"wk{c}", name=f"wk{c}")
                for c in range(NCC)]
        wv_c = [persist.tile([128, GD], BF16, tag=f"wv{c}", name=f"wv{c}")
                for c in range(NCC)]
        wp_m = [persist.tile([128, C], BF16, tag=f"wp{m}", name=f"wp{m}")
                for m in range(NMB)]

        ident = consts.tile([128, 128], BF16, tag="ident")
        make_identity(nc, ident)

        # ---- zero-fill pad regions (overlaps the initial DMAs) ----
        for hp in range(2):
            zbase = (1 - hp) * 64
            for m in range(NMB):
                nc.vector.memset(qTp[hp][m][zbase : zbase + 64, :], 0.0)
        nc.vector.memset(v_pad[:, :, :, HD + 1 :], 0.0)

        # ---- input DMAs ----
        for c in range(NCC):
            nc.sync.dma_start(out=wq_c[c], in_=wq[bass.ts(c, 128), :])
            nc.sync.dma_start(out=wk_c[c], in_=wk[bass.ts(c, 128), :])
            nc.sync.dma_start(out=wv_c[c], in_=wv[bass.ts(c, 128), :])
        for m in range(NMB):
            nc.sync.dma_start(out=wp_m[m], in_=wp[bass.ts(m, 128), :])

        bq_col = consts.tile([128, NMB], FP32, tag="bq_col")
        bk_col = consts.tile([128, NMB], FP32, tag="bk_col")
        for m in range(NMB):
            nc.sync.dma_start(out=bq_col[:, m : m + 1], in_=bq[bass.ts(m, 128)])
            nc.sync.dma_start(out=bk_col[:, m : m + 1], in_=bk[bass.ts(m, 128)])
        bv_bc = consts.tile([128, GD], FP32, tag="bv_bc")
        nc.sync.dma_start(out=bv_bc, in_=bv[None, :].partition_broadcast(128))
        # ones column of v_pad (after the memset in program order)
        for kc in range(NKC):
            nc.sync.dma_start(out=v_pad[:, kc, :, HD : HD + 1], in_=ones[:, :, None])

        # ---- prologue: linear-DMA x and transpose it on the PE ----
        with (
            tc.tile_pool(name="xn", bufs=4) as xn_pool,
            tc.tile_pool(name="trps", bufs=4, space="PSUM") as trps_pool,
        ):
            for i in range(NKC):        # 128-row chunk of x
                tb, tsub = divmod(i, 4)
                xn = xn_pool.tile([128, C], BF16, tag="xn")
                nc.sync.dma_start(out=xn, in_=x[bass.ts(i, 128), :])
                for c in range(NCC):
                    trp = trps_pool.tile([128, 128], BF16, tag="trp")
                    nc.tensor.transpose(trp, xn[:, bass.ts(c, 128)], ident)
                    nc.vector.tensor_copy(xtc[tb][c][:, bass.ts(tsub, 128)], trp)

        # ---- main pools: shared 2-bank pair pool + PV + proj ----
        with (
            tc.tile_pool(name="pT", bufs=6) as pT_pool,
            tc.tile_pool(name="rec", bufs=2) as rec_pool,
            tc.tile_pool(name="rbc", bufs=2) as rbc_pool,
            tc.tile_pool(name="osb", bufs=2) as osb_pool,
            tc.tile_pool(name="big", bufs=2, space="PSUM") as big_pool,
            tc.tile_pool(name="pvps", bufs=3, space="PSUM") as pvps_pool,
            tc.tile_pool(name="ops", bufs=1, space="PSUM") as ops_pool,
        ):
            def emit_qkv(tb):
                tcols = bass.ts(tb, 512)
                for m in range(NMB):
                    ps = big_pool.tile([128, 2, 512], FP32, tag="big")
                    for c in range(NCC):
                        nc.tensor.matmul(
                            ps[:, 0, :], wq_c[c][:, bass.ts(m, 128)], xtc[tb][c],
                            start=(c == 0), stop=(c == NCC - 1),
                        )
                    for c in range(NCC):
                        nc.tensor.matmul(
                            ps[:, 1, :], wk_c[c][:, bass.ts(m, 128)], xtc[tb][c],
                            start=(c == 0), stop=(c == NCC - 1),
                        )
                    nc.vector.tensor_scalar_add(
                        qTp[0][m][0:64, tcols], ps[0:64, 0, :], bq_col[0:64, m : m + 1]
                    )
                    nc.vector.tensor_scalar_add(
                        qTp[1][m][64:128, tcols], ps[64:128, 0, :],
                        bq_col[64:128, m : m + 1],
                    )
                    nc.vector.tensor_scalar_add(
                        kT_t[m][:, tcols], ps[:, 1, :], bk_col[:, m : m + 1]
                    )
                for tp in range(2):     # tsub pairs
                    ps = big_pool.tile([128, 2, 512], FP32, tag="big")
                    for j in range(2):
                        tsub = 2 * tp + j
                        for c in range(NCC):
                            nc.tensor.matmul(
                                ps[:, j, :],
                                xtc[tb][c][:, bass.ts(tsub, 128)], wv_c[c],
                                start=(c == 0), stop=(c == NCC - 1),
                            )
                    for j in range(2):
                        kc = tb * 4 + 2 * tp + j
                        vt = v_pad[:, kc, :, :]
                        nc.vector.tensor_add(
                            vt[:, :, 0:HD],
                            ps[:, j, :].rearrange("p (h d) -> p h d", h=HPG),
                            bv_bc.rearrange("p (h d) -> p h d", h=HPG),
                        )

            dens = {}

            def emit_attention(qb):
                kcmax = (qb + 1) * 4
                for m in range(NMB):
                    for hp in range(2):
                        pv = pvps_pool.tile([128, 512], FP32, tag="pvps", name="pvps")
                        for p in range(kcmax // 2):
                            sp = big_pool.tile([128, 2, 512], FP32, tag="big")
                            for j in range(2):
                                kc = 2 * p + j
                                nc.tensor.matmul(
                                    sp[:, j, :],
                                    kT_t[m][:, bass.ts(kc, 128)],
                                    qTp[hp][m][:, bass.ts(qb, 512)],
                                    start=True, stop=True,
                                )
                            pT2 = pT_pool.tile([128, 1024], BF16, tag="pT2")
                            nc.scalar.activation(
                                out=pT2,
                                in_=sp.rearrange("p a b -> p (a b)"),
                                func=mybir.ActivationFunctionType.Exp,
                                scale=1.0 / float(np.sqrt(HD)),
                            )
                            r0 = 2 * p - qb * 4
                            if r0 >= 0:
                                # keep key<=query on both 512-halves at once:
                                # iota = jl - il - 128*(r0 + j)
                                nc.gpsimd.affine_select(
                                    out=pT2, in_=pT2,
                                    compare_op=mybir.AluOpType.is_ge,
                                    fill=0.0,
                                    base=-128 * r0,
                                    channel_multiplier=-1,
                                    pattern=[[-128, 2], [1, 512]],
                                )
                            for j in range(2):
                                kc = 2 * p + j
                                nc.tensor.matmul(
                                    pv,
                                    v_pad[:, kc, 2 * m + hp, :],
                                    pT2[:, bass.ts(j, 512)],
                                    start=(kc == 0), stop=(kc == kcmax - 1),
                                )
                        if qb == 0 and hp == 0:
                            dens[m] = rec_pool.tile(
                                [64, 512], FP32, tag=f"den{m}", name=f"den{m}", bufs=1
                            )
                            nc.vector.memset(dens[m], 1.0)
                        # unnormalized y^T (bf16) + denominator row (at 0 / 32)
                        nc.vector.tensor_copy(
                            yT_t[m][bass.ts(hp, 64), bass.ts(qb, 512)], pv[0:HD, :]
                        )
                        nc.vector.tensor_copy(
                            dens[m][32 * hp : 32 * hp + 1, :], pv[HD : HD + 1, :]
                        )
                # normalize: cheap approx reciprocal, broadcast, in-place multiply
                for m in range(NMB):
                    denr = rec_pool.tile([64, 512], FP32, tag="denr", name="denr")
                    nc.vector.reciprocal_approx_fast(denr, dens[m])
                    for hp in range(2):
                        if hp == 0:
                            src_row = denr[0:1, :]
                        else:
                            dtmp = rec_pool.tile([1, 512], FP32, tag="dtmp", name="dtmp")
                            nc.vector.tensor_copy(dtmp, denr[32:33, :])
                            src_row = dtmp
                        rbc = rbc_pool.tile([128, 512], FP32, tag="rbc", name="rbc")
                        nc.gpsimd.partition_broadcast(rbc, src_row)
                        yt = yT_t[m][bass.ts(hp, 64), bass.ts(qb, 512)]
                        nc.vector.tensor_mul(yt, yt, rbc[bass.ts(hp, 64), :])

            def emit_proj(qb):
                for tsub in range(4):
                    tb16 = qb * 4 + tsub
                    for nb in range(C // 512):
                        ps = ops_pool.tile([128, 512], FP32, tag="ops")
                        for c in range(NMB):
                            nc.tensor.matmul(
                                ps,
                                yT_t[c][:, bass.ts(tb16, 128)],
                                wp_m[c][:, bass.ts(nb, 512)],
                                start=(c == 0), stop=(c == NMB - 1),
                            )
                        osb = osb_pool.tile([128, 512], FP32, tag="osb")
                        nc.vector.tensor_copy(osb, ps)
                        nc.sync.dma_start(
                            out=out[bass.ts(tb16, 128), bass.ts(nb, 512)], in_=osb
                        )

            # interleave: qkv(tb) feeds attention(qb=tb); proj lags one block
            emit_qkv(0)
            emit_attention(0)
            emit_qkv(1)
            emit_attention(1)
            emit_proj(0)
            emit_qkv(2)
            emit_attention(2)
            emit_proj(1)
            emit_qkv(3)
            emit_attention(3)
            emit_proj(2)
            emit_proj(3)

    nc.compile()
    return nc


_CACHE = {}


def _get_nc():
    if "nc" not in _CACHE:
        _CACHE["nc"] = build_nc()
    return _CACHE["nc"]


def _to_bf16(a):
    import ml_dtypes

    a = np.asarray(a, dtype=np.float32)
    return np.ascontiguousarray(a.astype(ml_dtypes.bfloat16))


def make_in_maps(x, w_qkv, b_qkv, w_proj):
    x = np.asarray(x, dtype=np.float32)
    w_qkv = np.asarray(w_qkv, dtype=np.float32)
    b_qkv = np.asarray(b_qkv, dtype=np.float32)
    in_maps = []
    for core in range(N_CORES):
        b, g = divmod(core, G)
        in_maps.append(
            {
                "x": _to_bf16(x[b]),
                "wq": _to_bf16(w_qkv[:, GD * g : GD * g + GD]),
                "wk": _to_bf16(w_qkv[:, C + GD * g : C + GD * g + GD]),
                "wv": _to_bf16(w_qkv[:, 2 * C + GD * g : 2 * C + GD * g + GD]),
                "bq": np.ascontiguousarray(b_qkv[GD * g : GD * g + GD]),
                "bk": np.ascontiguousarray(b_qkv[C + GD * g : C + GD * g + GD]),
                "bv": np.ascontiguousarray(b_qkv[2 * C + GD * g : 2 * C + GD * g + GD]),
                "wp": _to_bf16(np.asarray(w_proj, dtype=np.float32)[GD * g : GD * g + GD, :]),
                "ones": _to_bf16(np.ones((128, HPG), dtype=np.float32)),
            }
        )
    return in_maps


def _assemble(results, b_proj):
    y = np.empty((B, T, C), dtype=np.float32)
    for b in range(B):
        y[b] = results[G * b]["out"] + results[G * b + 1]["out"]
    y += np.asarray(b_proj, dtype=np.float32)[None, None, :]
    return y


def kernel(x, w_qkv, b_qkv, w_proj, b_proj):
    nc = _get_nc()
    in_maps = make_in_maps(x, w_qkv, b_qkv, w_proj)
    res = run_bass_kernel_spmd(nc, in_maps, list(range(N_CORES)))
    return _assemble(res.results, b_proj)
